# revision 7
# baseline (speedup 1.0000x reference)
"""Causal self-attention (causal-average variant) Bass kernel for 8 TRN2 cores.

Reference computation (B=4, T=2048, C=1024, fp32):
    v = x @ Wc.T                      # [B,T,C]
    y[b,t,:] = mean_{s<=t} v[b,s,:]   # causal averaging (the per-head split in
                                      # the reference is a no-op: the mask is
                                      # head-independent)
    out = y @ Wp.T                    # [B,T,C]

Algebraic restructuring: there is no nonlinearity between the two
projections, and the causal averaging acts on t while the projections act on
channels, so everything commutes:

    out = diag(1/(t+1)) @ cumsum_t(x) @ (Wp @ Wc).T

The two weight matrices fold into one W = Wp @ Wc on the host, the cumsum
moves onto x (DVE prefix-scan), and the 1/(t+1) scale lands on the t axis of
the output (fused per-partition scale in the PSUM drain). Per-core device
work is ONE 1024^3 bf16 matmul (27.4us PE floor, measured: LDWEIGHTS fully
hidden) plus the scan (~14.5us DVE, hidden) and 4MB DMA.

Sharding: 8 shards = (batch b in 0..3) x (sequence half j in 0..1), no
collectives. Core gets x[b, 1024j:1024(j+1)].T with the first-half column sum
folded into row 0 for j=1 (cumsum of the local block then equals the global
prefix sum).

Pipeline structure (measured on HW; see NOTES.md):
  - W = (Wp@Wc).T stays RESIDENT in SBUF (loaded once in the prologue);
    re-DMAing it every iteration interferes with the x/out streams.
  - MMs run KT-OUTER: for each half (4 tt-groups in 4 [128,1024] psum
    tiles = all 8 banks), the kt loop is outermost, so the first MM of a
    body depends only on the kt=0 scan — the PE chases the scan chain
    tile-by-tile instead of waiting for the whole scan block. This
    recovered the ~3-8us the body-level scan->MM dependency was costing;
    body-level prefetch/stagger schemes (pf, staggered_reset) regressed.
  - Drains (ACT, per-partition 1/(t+1) scale, bf16 downcast) chase each
    half's accumulation stops; out-DMA on the gpsimd ring.

Engine budget per core/iter: PE 128 matmuls x 512 cols = 65536 cyc @2.4GHz
= 27.3us (the floor for 1.07 GMAC); DVE scans ~14.5us; ACT ~8.5us; DMA 4MB
in+out ~11us. Everything but PE hides under the matmul in steady state.
"""
import sys

sys.path.insert(0, "/opt/trn_rl_repo")

import numpy as np
import ml_dtypes

import concourse.bass as bass  # noqa: F401  (import keeps bass registered)
import concourse.tile as tile
from concourse import bacc, mybir
from concourse.bass_utils import run_bass_kernel_spmd

P = 128          # partitions
TH = 1024        # sequence half per core
C = 1024         # channels
NT = TH // P     # 8 t-tiles
NKT = C // P     # 8 k-tiles
ND = 2           # d-halves (512-wide matmul moving blocks)
NB = C // ND     # 512
CORES = list(range(8))

BF = mybir.dt.bfloat16
F32 = mybir.dt.float32

_CACHE = {}


def _build(repeat=1, bench=False, wu=0, x_bufs=2, w_bufs=2, c_bufs=2,
           o_bufs=4, odma="gpsimd", wdma="scalar", sch=1,
           hwloop=None, ucand=(4, 2, 1), wres=True, mm="ktouter"):
    nc = bacc.Bacc("TRN2", target_bir_lowering=False, debug=False, num_devices=8)
    # DRAM layouts chosen so every DMA is a contiguous slice. In bench mode
    # the big tensors are Internal (uninitialized garbage — DMA and matmul
    # timing is data-independent) so per-call transfer is tiny.
    kin = "Internal" if bench else "ExternalInput"
    kout = "Internal" if bench else "ExternalOutput"
    x_d = nc.dram_tensor("xt", [NKT, P, TH], BF, kind=kin)   # [kt, p(k), t]
    w_d = nc.dram_tensor("wt", [NKT, P, C], BF, kind=kin)    # [kt, p(k), d]
    iv_d = nc.dram_tensor("iv", [P, NT], F32, kind=kin)      # 1/(t_glob+1)
    o_d = nc.dram_tensor("o", [2, P, 4 * C], BF, kind=kout)  # [half, p(t), q*C+d]
    if bench:
        din_d = nc.dram_tensor("din", [P, 8], F32, kind="ExternalInput")
        dout_d = nc.dram_tensor("dout", [P, 8], F32, kind="ExternalOutput")

    add = mybir.AluOpType.add

    with tile.TileContext(nc) as tc:
        with (
            tc.tile_pool(name="x", bufs=1) as x_pool,
            tc.tile_pool(name="w", bufs=1) as w_pool,
            tc.tile_pool(name="c", bufs=1) as c_pool,
            tc.tile_pool(name="o", bufs=1) as o_pool,
            tc.tile_pool(name="m", bufs=1) as m_pool,
            tc.tile_pool(name="ps", bufs=1, space="PSUM") as ps_pool,
        ):

            # Loop-invariant prologue: averaging denominators, the scan's
            # zero operand, and (wres) the folded weight — all loaded once.
            iv_t = m_pool.tile([P, NT], F32, tag="iv", name="iv_t", bufs=1)
            nc.sync.dma_start(iv_t[:], iv_d[:])
            zero_t = m_pool.tile([P, TH], BF, tag="z", name="zero_t", bufs=1)
            nc.gpsimd.memset(zero_t[:], 0.0)
            pw_ts = None
            if wres:
                pw_ts = {}
                for kt in range(NKT):
                    pw_ts[kt] = w_pool.tile([P, C], BF, tag=f"w{kt}",
                                            name=f"w{kt}", bufs=1)
                    getattr(nc, wdma).dma_start(pw_ts[kt][:], w_d[kt])
            # PE warmup: dummy matmuls with no DMA deps cover the initial
            # HAM clock-gate ramp on the single-shot path. Reuses the ps0
            # psum tag so the 4 compute psum tiles still fit in 8 banks.
            if wu:
                wu_t = m_pool.tile([P, NB], BF, tag="wu", name="wu_t", bufs=1)
                nc.gpsimd.memset(wu_t[:], 0.0)
                wu_ps = ps_pool.tile([P, C], F32, tag="ps0", name="wu_ps",
                                     bufs=1)
                for _ in range(wu):
                    nc.tensor.matmul(wu_ps[:, :NB], wu_t[:, :P], wu_t[:],
                                     start=True, stop=True)

            def prefetch():
                # Load + scan stage. x DMAs on the SP ring; w (non-wres
                # mode only) on a second ring so both streams run
                # concurrently.
                x_ts, c_ts = {}, {}
                for kt in range(NKT):
                    x_ts[kt] = x_pool.tile([P, TH], BF, tag=f"x{kt}",
                                           name=f"x{kt}", bufs=x_bufs)
                    nc.sync.dma_start(x_ts[kt][:], x_d[kt])
                if wres:
                    w_ts = pw_ts
                else:
                    w_ts = {}
                    for kt in range(NKT):
                        w_ts[kt] = w_pool.tile([P, C], BF, tag=f"w{kt}",
                                               name=f"w{kt}", bufs=w_bufs)
                        getattr(nc, wdma).dma_start(w_ts[kt][:], w_d[kt])

                # xc[kt] = cumsum over t (free dim) of x[kt]; fp32 scan state,
                # bf16 output feeds the PE as lhsT. For j=1 cores the
                # first-half carry is pre-folded into x row t=0 on the host,
                # so initial=0 still yields the global prefix sum.
                SC = TH // sch
                for kt in range(NKT):
                    c_ts[kt] = c_pool.tile([P, TH], BF, tag=f"c{kt}",
                                           name=f"c{kt}", bufs=c_bufs)
                for ch in range(sch):
                    sl = slice(ch * SC, (ch + 1) * SC)
                    for kt in range(NKT):
                        nc.vector.tensor_tensor_scan(
                            c_ts[kt][:, sl], x_ts[kt][:, sl], zero_t[:, :SC],
                            0.0 if ch == 0 else c_ts[kt][:, ch * SC - 1:ch * SC],
                            add, add)
                return w_ts, c_ts

            def compute_ktouter(state):
                # kt-outer MM order: per half, 4 tt-groups accumulate across
                # kt in 4 [P,C] psum tiles (all 8 banks); groups interleave
                # in the PE stream (different banks — legal). The first MM
                # of a body depends only on the kt=0 scan.
                w_ts, c_ts = state
                for half in range(2):
                    ps = {}
                    for q in range(4):
                        ps[q] = ps_pool.tile([P, C], F32, tag=f"ps{q}",
                                             name=f"ps{q}", bufs=1)
                    for kt in range(NKT):
                        for q in range(4):
                            tt = half * 4 + q
                            for dh in range(ND):
                                nc.tensor.matmul(
                                    ps[q][:, dh * NB:(dh + 1) * NB],
                                    c_ts[kt][:, tt * P:(tt + 1) * P],
                                    w_ts[kt][:, dh * NB:(dh + 1) * NB],
                                    start=(kt == 0), stop=(kt == NKT - 1))
                    # One [128, 4096] drain tile per half -> a single 1MB
                    # out-DMA (4x fewer descriptor generations; measured win
                    # over per-tt 256KB DMAs).
                    oh_t = o_pool.tile([P, 4 * C], BF, tag="oh", name="oh_t",
                                       bufs=2)
                    for q in range(4):
                        tt = half * 4 + q
                        nc.scalar.mul(oh_t[:, q * C:(q + 1) * C], ps[q][:],
                                      iv_t[:, tt:tt + 1])
                    getattr(nc, odma).dma_start(o_d[half], oh_t[:])

            def compute_ttouter(state):
                # Original order: one psum tile per tt, kt inner.
                w_ts, c_ts = state
                for tt in range(NT):
                    psum = ps_pool.tile([P, C], F32, tag="ps", name="psum",
                                        bufs=3)
                    for dh in range(ND):
                        for kt in range(NKT):
                            nc.tensor.matmul(
                                psum[:, dh * NB:(dh + 1) * NB],
                                c_ts[kt][:, tt * P:(tt + 1) * P],
                                w_ts[kt][:, dh * NB:(dh + 1) * NB],
                                start=(kt == 0), stop=(kt == NKT - 1))
                    o_t = o_pool.tile([P, C], BF, tag="o", bufs=o_bufs)
                    nc.scalar.mul(o_t[:], psum[:], iv_t[:, tt:tt + 1])
                    getattr(nc, odma).dma_start(
                        o_d[tt // 4, :, (tt % 4) * C:(tt % 4 + 1) * C], o_t[:])

            def body():
                state = prefetch()
                if mm == "ktouter":
                    compute_ktouter(state)
                else:
                    compute_ttouter(state)

            if hwloop is None:
                use_loop = bench and repeat > 1
            else:
                use_loop = hwloop and repeat > 1
            if use_loop:
                # For_i ends every iteration with an all-engine barrier +
                # semaphore reset; unroll U bodies inside the loop so tag
                # rotation pipelines them and the barrier cost amortizes.
                # (Measured: U=2/4/6 equivalent; staggered_reset regresses.)
                U = max(u for u in ucand if repeat % u == 0)
                with tc.For_i(0, repeat // U, 1):
                    for _u in range(U):
                        body()
            else:
                for _rep in range(repeat):
                    body()
            if bench:
                with tc.tile_pool(name="dummy", bufs=1) as d_pool:
                    d_t = d_pool.tile([P, 8], F32)
                    nc.sync.dma_start(d_t[:], din_d[:])
                    nc.sync.dma_start(dout_d[:], d_t[:])

    nc.compile()
    return nc


def _get_program(repeat=1, bench=False, **kw):
    if bench:
        kw.setdefault("wu", 0)
    else:
        kw.setdefault("wu", 20)
    key = ("nc", repeat, bench, tuple(sorted(kw.items())))
    if key not in _CACHE:
        _CACHE[key] = _build(repeat, bench, **kw)
    return _CACHE[key]


def _consts():
    # 1/(t_global+1) laid out [p(t), tt] per sequence-half j.
    if "iv" not in _CACHE:
        ivs = []
        for j in range(2):
            tg = (TH * j + np.arange(TH, dtype=np.float32)).reshape(NT, P)
            ivs.append(np.ascontiguousarray((1.0 / (tg + 1.0)).T))  # [p, tt]
        _CACHE["iv"] = ivs
    return _CACHE["iv"]


def _prep_inputs(x, Wc, Wp):
    x = np.ascontiguousarray(np.asarray(x, dtype=np.float32))
    Wc = np.asarray(Wc, dtype=np.float32)
    Wp = np.asarray(Wp, dtype=np.float32)

    # W = Wp @ Wc folds both projections; device consumes W.T = Wc.T @ Wp.T
    # as [p(k), d] tiles.
    wT = np.ascontiguousarray(Wc.T @ Wp.T)                   # [k, d]
    w_in = wT.reshape(NKT, P, C).astype(ml_dtypes.bfloat16)  # [kt, p(k), d]

    ivs = _consts()

    in_maps = []
    for core in CORES:
        b, j = divmod(core, 2)
        xs = x[b, TH * j:TH * (j + 1)].copy()
        if j == 1:
            xs[0] += x[b, :TH].sum(axis=0)
        xt = np.ascontiguousarray(xs.T).reshape(NKT, P, TH)  # [kt, p(k), t]
        in_maps.append({"xt": xt.astype(ml_dtypes.bfloat16),
                        "wt": w_in, "iv": ivs[j]})
    return in_maps


def _run(x, Wc, Wp, trace=False, repeat=1):
    nc = _get_program(repeat)
    in_maps = _prep_inputs(x, Wc, Wp)
    res = run_bass_kernel_spmd(nc, in_maps, CORES, trace=trace)
    B = np.asarray(x).shape[0]
    out = np.empty((B, 2 * TH, C), dtype=np.float32)
    for core in CORES:
        b, j = divmod(core, 2)
        o = res.results[core]["o"]                 # [half, p(t), q*C+d] bf16
        oh = np.asarray(o).reshape(2, P, 4, C).transpose(0, 2, 1, 3)
        out[b, TH * j:TH * (j + 1)] = oh.reshape(TH, C).astype(np.float32)
    return out, res


def kernel(x, Wc, Wp):
    out, _ = _run(x, Wc, Wp, trace=False)
    return out


# revision 10
# speedup vs baseline: 1.1184x; 1.1184x over previous
"""Causal self-attention (causal-average variant) Bass kernel for 8 TRN2 cores.

Reference computation (B=4, T=2048, C=1024, fp32):
    v = x @ Wc.T                      # [B,T,C]
    y[b,t,:] = mean_{s<=t} v[b,s,:]   # causal averaging (the per-head split in
                                      # the reference is a no-op: the mask is
                                      # head-independent)
    out = y @ Wp.T                    # [B,T,C]

Algebraic restructuring: there is no nonlinearity between the two
projections, and the causal averaging acts on t while the projections act on
channels, so everything commutes:

    out = diag(1/(t+1)) @ cumsum_t(x) @ (Wp @ Wc).T

The two weight matrices fold into one W = Wp @ Wc on the host, the cumsum
moves onto x (DVE prefix-scan), and the 1/(t+1) scale lands on the t axis of
the output (fused per-partition scale in the PSUM drain). Per-core device
work is ONE 1024^3 bf16 matmul (27.4us PE floor, measured: LDWEIGHTS fully
hidden) plus the scan (~14.5us DVE, hidden) and 4MB DMA.

Sharding: 8 shards = (batch b in 0..3) x (sequence half j in 0..1), no
collectives. Core gets x[b, 1024j:1024(j+1)].T with the first-half column sum
folded into row 0 for j=1 (cumsum of the local block then equals the global
prefix sum).

Pipeline structure (measured on HW; see NOTES.md):
  - W = (Wp@Wc).T stays RESIDENT in SBUF (loaded once in the prologue);
    re-DMAing it every iteration interferes with the x/out streams.
  - MMs run KT-OUTER: for each half (4 tt-groups in 4 [128,1024] psum
    tiles = all 8 banks), the kt loop is outermost, so the first MM of a
    body depends only on the kt=0 scan — the PE chases the scan chain
    tile-by-tile instead of waiting for the whole scan block. This
    recovered the ~3-8us the body-level scan->MM dependency was costing;
    body-level prefetch/stagger schemes (pf, staggered_reset) regressed.
  - Drains (ACT, per-partition 1/(t+1) scale, bf16 downcast) chase each
    half's accumulation stops; out-DMA on the gpsimd ring.

Engine budget per core/iter: PE 128 matmuls x 512 cols = 65536 cyc @2.4GHz
= 27.3us (the floor for 1.07 GMAC); DVE scans ~14.5us; ACT ~8.5us; DMA 4MB
in+out ~11us. Everything but PE hides under the matmul in steady state.
"""
import sys

sys.path.insert(0, "/opt/trn_rl_repo")

import numpy as np
import ml_dtypes

import concourse.bass as bass  # noqa: F401  (import keeps bass registered)
import concourse.tile as tile
from concourse import bacc, mybir
from concourse.bass_utils import run_bass_kernel_spmd

P = 128          # partitions
TH = 1024        # sequence half per core
C = 1024         # channels
NT = TH // P     # 8 t-tiles
NKT = C // P     # 8 k-tiles
ND = 2           # d-halves (512-wide matmul moving blocks)
NB = C // ND     # 512
CORES = list(range(8))

BF = mybir.dt.bfloat16
F32 = mybir.dt.float32

_CACHE = {}


def _build(repeat=1, bench=False, wu=0, x_bufs=2, w_bufs=2, c_bufs=2,
           o_bufs=4, odma="gpsimd", wdma="scalar", sch=1,
           hwloop=None, ucand=(4, 2, 1), wres=True, mm="ktouter"):
    nc = bacc.Bacc("TRN2", target_bir_lowering=False, debug=False, num_devices=8)
    # DRAM layouts chosen so every DMA is a contiguous slice. In bench mode
    # the big tensors are Internal (uninitialized garbage — DMA and matmul
    # timing is data-independent) so per-call transfer is tiny.
    kin = "Internal" if bench else "ExternalInput"
    kout = "Internal" if bench else "ExternalOutput"
    x_d = nc.dram_tensor("xt", [P, NKT * TH], BF, kind=kin)  # [p, kt*TH+t]
    w_d = nc.dram_tensor("wt", [NKT, P, C], BF, kind=kin)    # [kt, p(k), d]
    iv_d = nc.dram_tensor("iv", [P, NT], F32, kind=kin)      # 1/(t_glob+1)
    o_d = nc.dram_tensor("o", [2, P, 4 * C], BF, kind=kout)  # [half, p(t), q*C+d]
    if bench:
        din_d = nc.dram_tensor("din", [P, 8], F32, kind="ExternalInput")
        dout_d = nc.dram_tensor("dout", [P, 8], F32, kind="ExternalOutput")

    add = mybir.AluOpType.add

    with tile.TileContext(nc) as tc:
        with (
            tc.tile_pool(name="x", bufs=1) as x_pool,
            tc.tile_pool(name="w", bufs=1) as w_pool,
            tc.tile_pool(name="c", bufs=1) as c_pool,
            tc.tile_pool(name="o", bufs=1) as o_pool,
            tc.tile_pool(name="m", bufs=1) as m_pool,
            tc.tile_pool(name="ps", bufs=1, space="PSUM") as ps_pool,
        ):

            # Loop-invariant prologue: averaging denominators, the scan's
            # zero operand, and (wres) the folded weight — all loaded once.
            iv_t = m_pool.tile([P, NT], F32, tag="iv", name="iv_t", bufs=1)
            nc.sync.dma_start(iv_t[:], iv_d[:])
            zero_t = m_pool.tile([P, TH], BF, tag="z", name="zero_t", bufs=1)
            nc.gpsimd.memset(zero_t[:], 0.0)
            pw_ts = None
            if wres:
                pw_ts = {}
                for kt in range(NKT):
                    pw_ts[kt] = w_pool.tile([P, C], BF, tag=f"w{kt}",
                                            name=f"w{kt}", bufs=1)
                    getattr(nc, wdma).dma_start(pw_ts[kt][:], w_d[kt])
            # PE warmup: dummy matmuls with no DMA deps cover the initial
            # HAM clock-gate ramp on the single-shot path. Reuses the ps0
            # psum tag so the 4 compute psum tiles still fit in 8 banks.
            if wu:
                wu_t = m_pool.tile([P, NB], BF, tag="wu", name="wu_t", bufs=1)
                nc.gpsimd.memset(wu_t[:], 0.0)
                wu_ps = ps_pool.tile([P, C], F32, tag="ps0", name="wu_ps",
                                     bufs=1)
                for _ in range(wu):
                    nc.tensor.matmul(wu_ps[:, :NB], wu_t[:, :P], wu_t[:],
                                     start=True, stop=True)

            def prefetch():
                # Load + scan stage. x DMAs on the SP ring; w (non-wres
                # mode only) on a second ring so both streams run
                # concurrently.
                # x arrives host-permuted as [p, kt*TH+t]: 4 x 512KB DMAs
                # (quarter the issue/sem count of per-tile DMAs; 2-tile
                # granularity keeps the scan chain pipelined).
                x_ts, c_ts = {}, {}
                xall = x_pool.tile([P, NKT * TH], BF, tag="xa", name="xa",
                                   bufs=x_bufs)
                wd = NKT * TH // 4
                for h in range(4):
                    nc.sync.dma_start(xall[:, h * wd:(h + 1) * wd],
                                      x_d[:, h * wd:(h + 1) * wd])
                for kt in range(NKT):
                    x_ts[kt] = xall[:, kt * TH:(kt + 1) * TH]
                if wres:
                    w_ts = pw_ts
                else:
                    w_ts = {}
                    for kt in range(NKT):
                        w_ts[kt] = w_pool.tile([P, C], BF, tag=f"w{kt}",
                                               name=f"w{kt}", bufs=w_bufs)
                        getattr(nc, wdma).dma_start(w_ts[kt][:], w_d[kt])

                # xc[kt] = cumsum over t (free dim) of x[kt]; fp32 scan state,
                # bf16 output feeds the PE as lhsT. For j=1 cores the
                # first-half carry is pre-folded into x row t=0 on the host,
                # so initial=0 still yields the global prefix sum.
                SC = TH // sch
                for kt in range(NKT):
                    c_ts[kt] = c_pool.tile([P, TH], BF, tag=f"c{kt}",
                                           name=f"c{kt}", bufs=c_bufs)
                for ch in range(sch):
                    sl = slice(ch * SC, (ch + 1) * SC)
                    for kt in range(NKT):
                        nc.vector.tensor_tensor_scan(
                            c_ts[kt][:, sl], x_ts[kt][:, sl], zero_t[:, :SC],
                            0.0 if ch == 0 else c_ts[kt][:, ch * SC - 1:ch * SC],
                            add, add)
                return w_ts, c_ts

            def compute_ktouter(state):
                # kt-outer MM order: per half, 4 tt-groups accumulate across
                # kt in 4 [P,C] psum tiles (all 8 banks); groups interleave
                # in the PE stream (different banks — legal). The first MM
                # of a body depends only on the kt=0 scan.
                w_ts, c_ts = state
                for half in range(2):
                    ps = {}
                    for q in range(4):
                        ps[q] = ps_pool.tile([P, C], F32, tag=f"ps{q}",
                                             name=f"ps{q}", bufs=1)
                    for kt in range(NKT):
                        for q in range(4):
                            tt = half * 4 + q
                            for dh in range(ND):
                                nc.tensor.matmul(
                                    ps[q][:, dh * NB:(dh + 1) * NB],
                                    c_ts[kt][:, tt * P:(tt + 1) * P],
                                    w_ts[kt][:, dh * NB:(dh + 1) * NB],
                                    start=(kt == 0), stop=(kt == NKT - 1))
                    # One [128, 4096] drain tile per half -> a single 1MB
                    # out-DMA (4x fewer descriptor generations; measured win
                    # over per-tt 256KB DMAs).
                    oh_t = o_pool.tile([P, 4 * C], BF, tag="oh", name="oh_t",
                                       bufs=2)
                    for q in range(4):
                        tt = half * 4 + q
                        nc.scalar.mul(oh_t[:, q * C:(q + 1) * C], ps[q][:],
                                      iv_t[:, tt:tt + 1])
                    getattr(nc, odma).dma_start(o_d[half], oh_t[:])

            def compute_ttouter(state):
                # Original order: one psum tile per tt, kt inner.
                w_ts, c_ts = state
                for tt in range(NT):
                    psum = ps_pool.tile([P, C], F32, tag="ps", name="psum",
                                        bufs=3)
                    for dh in range(ND):
                        for kt in range(NKT):
                            nc.tensor.matmul(
                                psum[:, dh * NB:(dh + 1) * NB],
                                c_ts[kt][:, tt * P:(tt + 1) * P],
                                w_ts[kt][:, dh * NB:(dh + 1) * NB],
                                start=(kt == 0), stop=(kt == NKT - 1))
                    o_t = o_pool.tile([P, C], BF, tag="o", bufs=o_bufs)
                    nc.scalar.mul(o_t[:], psum[:], iv_t[:, tt:tt + 1])
                    getattr(nc, odma).dma_start(
                        o_d[tt // 4, :, (tt % 4) * C:(tt % 4 + 1) * C], o_t[:])

            def body():
                state = prefetch()
                if mm == "ktouter":
                    compute_ktouter(state)
                else:
                    compute_ttouter(state)

            if hwloop is None:
                use_loop = bench and repeat > 1
            else:
                use_loop = hwloop and repeat > 1
            if use_loop:
                # For_i ends every iteration with an all-engine barrier +
                # semaphore reset; unroll U bodies inside the loop so tag
                # rotation pipelines them and the barrier cost amortizes.
                # (Measured: U=2/4/6 equivalent; staggered_reset regresses.)
                U = max(u for u in ucand if repeat % u == 0)
                with tc.For_i(0, repeat // U, 1):
                    for _u in range(U):
                        body()
            else:
                for _rep in range(repeat):
                    body()
            if bench:
                with tc.tile_pool(name="dummy", bufs=1) as d_pool:
                    d_t = d_pool.tile([P, 8], F32)
                    nc.sync.dma_start(d_t[:], din_d[:])
                    nc.sync.dma_start(dout_d[:], d_t[:])

    nc.compile()
    return nc


def _get_program(repeat=1, bench=False, **kw):
    if bench:
        kw.setdefault("wu", 0)
    else:
        kw.setdefault("wu", 20)
    key = ("nc", repeat, bench, tuple(sorted(kw.items())))
    if key not in _CACHE:
        _CACHE[key] = _build(repeat, bench, **kw)
    return _CACHE[key]


def _consts():
    # 1/(t_global+1) laid out [p(t), tt] per sequence-half j.
    if "iv" not in _CACHE:
        ivs = []
        for j in range(2):
            tg = (TH * j + np.arange(TH, dtype=np.float32)).reshape(NT, P)
            ivs.append(np.ascontiguousarray((1.0 / (tg + 1.0)).T))  # [p, tt]
        _CACHE["iv"] = ivs
    return _CACHE["iv"]


def _prep_inputs(x, Wc, Wp):
    x = np.ascontiguousarray(np.asarray(x, dtype=np.float32))
    Wc = np.asarray(Wc, dtype=np.float32)
    Wp = np.asarray(Wp, dtype=np.float32)

    # W = Wp @ Wc folds both projections; device consumes W.T = Wc.T @ Wp.T
    # as [p(k), d] tiles.
    wT = np.ascontiguousarray(Wc.T @ Wp.T)                   # [k, d]
    w_in = wT.reshape(NKT, P, C).astype(ml_dtypes.bfloat16)  # [kt, p(k), d]

    ivs = _consts()

    in_maps = []
    for core in CORES:
        b, j = divmod(core, 2)
        xs = x[b, TH * j:TH * (j + 1)].copy()
        if j == 1:
            xs[0] += x[b, :TH].sum(axis=0)
        xt = np.ascontiguousarray(xs.T).reshape(NKT, P, TH)  # [kt, p(k), t]
        xt_big = np.ascontiguousarray(
            xt.transpose(1, 0, 2).reshape(P, NKT * TH))      # [p, kt*TH+t]
        in_maps.append({"xt": xt_big.astype(ml_dtypes.bfloat16),
                        "wt": w_in, "iv": ivs[j]})
    return in_maps


def _run(x, Wc, Wp, trace=False, repeat=1):
    nc = _get_program(repeat)
    in_maps = _prep_inputs(x, Wc, Wp)
    res = run_bass_kernel_spmd(nc, in_maps, CORES, trace=trace)
    B = np.asarray(x).shape[0]
    out = np.empty((B, 2 * TH, C), dtype=np.float32)
    for core in CORES:
        b, j = divmod(core, 2)
        o = res.results[core]["o"]                 # [half, p(t), q*C+d] bf16
        oh = np.asarray(o).reshape(2, P, 4, C).transpose(0, 2, 1, 3)
        out[b, TH * j:TH * (j + 1)] = oh.reshape(TH, C).astype(np.float32)
    return out, res


def kernel(x, Wc, Wp):
    out, _ = _run(x, Wc, Wp, trace=False)
    return out
